# revision 2
# baseline (speedup 1.0000x reference)
"""Gaussian-mixture log-likelihood kernel for 8 Trainium2 NeuronCores.

Math: ll_i = logsumexp_j( -0.5 x_i^T A_j x_i + x_i^T m_j + bias_j ) - C
with A_j = S_j S_j^T.  The quadratic form is a single contraction of 564
"lift" rows per point against a [564, K] parameter matrix: 496 unique
symmetric-pair products, 32 squares, 32 linear rows, 3 bias rows (hi/mid/lo
split) and 1 zero pad.  Lift rows and parameters are stored in fp8-e4m3 with
per-row-type power-of-two scales folded oppositely into the two factors, so
the PE runs DoubleRow matmuls (256-row contraction per instruction, 2x
column rate): 3 matmuls per 128-point tile instead of 5 f16 ones.

The lift planes are packed host-side (cheap: O(N D^2) vs the device's
O(N K D^2) contraction) and shipped as fp8, which also removes the
shuffle/multiply traffic on DVE/Pool.  Per 1024-point slab the device does
24 DoubleRow matmuls -> one [128, 2048] Exp activation (PSUM spanning 4
banks) -> one segmented f16 add-reduce over K.  A global shift C (folded
into the bias rows) makes exp() safe without a per-point max; a single Ln +
bias-add finishes all 16384 points per core.

Sharding: data-parallel over points, 16384 points/core; K-sized parameters
are replicated (precomputed on host in float64 -- tiny vs the N*K work).
"""

import sys

sys.path.insert(0, "/opt/trn_rl_repo")

import numpy as np
import ml_dtypes

import concourse.bass as bass
import concourse.bacc as bacc
import concourse.mybir as mybir
from concourse import bass_utils
from concourse.bass_interp import get_hw_module
from concourse.tile import TileContext

N, K, D = 131072, 256, 32
NCORES = 8
NC_PTS = N // NCORES            # 16384 points per core
NTILES = NC_PTS // 128          # 128 output columns
DSLAB = 2048                    # points per DMA slab
NDMA = NC_PTS // DSLAB          # 8
PSLAB = 1024                    # points per PSUM slab (8 tiles x 256 K)
F32 = mybir.dt.float32
F16 = mybir.dt.float16
F8 = mybir.dt.float8e4

NROWS = 564                     # 512 (chunks A,B) + 52 (chunk C)
CROWS = 26                      # chunk C pair rows

SCL_P = 16.0                    # off-diagonal product rows
SCL_D = 4.0                     # diagonal (square) rows
SCL_L = 4.0                     # linear rows

_CACHE = {}


def _build(nc):
    lab = nc.dram_tensor("lab", [128, 4, NC_PTS], F8, kind="ExternalInput").ap()
    lc = nc.dram_tensor("lc", [CROWS, 2, NC_PTS], F8, kind="ExternalInput").ap()
    bab = nc.dram_tensor("bab", [128, 4, K], F8, kind="ExternalInput").ap()
    bc = nc.dram_tensor("bc", [CROWS, 2, K], F8, kind="ExternalInput").ap()
    consts = nc.dram_tensor("consts", [128, 1], F32, kind="ExternalInput").ap()
    out = nc.dram_tensor("out", [128, NTILES], F32, kind="ExternalOutput").ap()

    DR = mybir.MatmulPerfMode.DoubleRow

    with TileContext(nc) as tc:
        with (
            tc.tile_pool(name="par", bufs=1) as par_pool,
            tc.tile_pool(name="src", bufs=3) as src_pool,
            tc.tile_pool(name="eps", bufs=3) as eps_pool,
            tc.tile_pool(name="acc", bufs=1) as acc_pool,
            tc.tile_pool(name="psum", bufs=2, space="PSUM") as psum_pool,
        ):
            bab_t = par_pool.tile([128, 4, K], F8, tag="bab")
            bc_t = par_pool.tile([CROWS, 2, K], F8, tag="bc")
            negC = par_pool.tile([128, 1], F32, tag="negC")
            nc.sync.dma_start(out=bab_t[:, :, :], in_=bab[:, :, :])
            nc.sync.dma_start(out=bc_t[:, :, :], in_=bc[:, :, :])
            nc.sync.dma_start(out=negC[:, :], in_=consts[:, :])

            s_all = acc_pool.tile([128, NTILES], F16, tag="s_all")
            ll_all = acc_pool.tile([128, NTILES], F32, tag="ll_all")

            for dslab in range(NDMA):
                lo = dslab * DSLAB
                lab_t = src_pool.tile([128, 4, DSLAB], F8, tag="lab")
                lc_t = src_pool.tile([CROWS, 2, DSLAB], F8, tag="lc")
                nc.sync.dma_start(out=lab_t[:, :, :],
                                  in_=lab[:, :, lo:lo + DSLAB])
                nc.sync.dma_start(out=lc_t[:, :, :],
                                  in_=lc[:, :, lo:lo + DSLAB])

                for h in range(DSLAB // PSLAB):
                    ps = psum_pool.tile([128, 8 * K], F32, tag="ps")
                    for u in range(PSLAB // 128):
                        ts = slice(h * PSLAB + u * 128,
                                   h * PSLAB + (u + 1) * 128)
                        nc.tensor.matmul(out=ps[:, K * u:K * (u + 1)],
                                         lhsT=lab_t[:, 0:2, ts],
                                         rhs=bab_t[:, 0:2, :],
                                         start=True, stop=False, perf_mode=DR)
                        nc.tensor.matmul(out=ps[:, K * u:K * (u + 1)],
                                         lhsT=lab_t[:, 2:4, ts],
                                         rhs=bab_t[:, 2:4, :],
                                         start=False, stop=False, perf_mode=DR)
                        nc.tensor.matmul(out=ps[:, K * u:K * (u + 1)],
                                         lhsT=lc_t[:, :, ts],
                                         rhs=bc_t[:, :, :],
                                         start=False, stop=True, perf_mode=DR)
                    e_t = eps_pool.tile([128, 8, K], F16, tag="e")
                    nc.scalar.activation(
                        out=e_t[:, :, :],
                        in_=ps[:, :].rearrange("p (a b) -> p a b", a=8),
                        func=mybir.ActivationFunctionType.Exp)
                    col = (dslab * DSLAB + h * PSLAB) // 128
                    with nc.allow_low_precision(reason="f16 partial sums feed"
                                                " a log; rel err ~1e-3"):
                        nc.vector.tensor_reduce(out=s_all[:, col:col + 8],
                                                in_=e_t[:, :, :],
                                                axis=mybir.AxisListType.X,
                                                op=mybir.AluOpType.add)

            nc.scalar.activation(out=ll_all[:, :], in_=s_all[:, :],
                                 func=mybir.ActivationFunctionType.Ln)
            nc.vector.tensor_scalar_add(out=ll_all[:, :], in0=ll_all[:, :],
                                        scalar1=negC[:, 0:1])
            nc.sync.dma_start(out=out[:, :], in_=ll_all[:, :])
    return nc


def _get_module():
    if "nc" not in _CACHE:
        nc = bacc.Bacc("TRN2", target_bir_lowering=False, debug=False,
                       num_devices=NCORES)
        _build(nc)
        nc.compile()
        nc.m = get_hw_module(nc.m)
        _CACHE["nc"] = nc
    return _CACHE["nc"]


def _host_params(centers, covs_inv_sqrt, weights, threshold):
    """Per-cluster parameter rows (B) in fp8, plus the global shift C.

    Returns (Brows_f32 [NROWS, K] already scaled, C) -- quantization to fp8
    happens in kernel() alongside the lift planes.
    """
    S = covs_inv_sqrt.astype(np.float64)
    w = np.abs(weights.astype(np.float64))
    cp = w / (w.sum() + 1e-30)
    A = np.einsum("kde,kfe->kdf", S, S)
    _, logdetS = np.linalg.slogdet(S)
    logcoef = np.log(np.maximum(cp, 1e-300)) + logdetS
    cen = centers.astype(np.float64)
    m = np.einsum("kde,ke->kd", A, cen)
    t_cAc = np.einsum("kd,kd->k", m, cen)
    thr = float(threshold[0])
    C = 4.0 - (logcoef.max() - thr)
    bias = logcoef - 0.5 * t_cAc - thr + C

    Brows = np.zeros((NROWS, K), np.float64)
    r = 0
    for o in range(1, 16):                      # 480 off-diag product rows
        for i in range(32):
            Brows[r] = -A[:, i, (i + o) % 32] * SCL_P
            r += 1
    for i in range(16):                         # 16 distance-16 pairs
        Brows[r] = -A[:, i, i + 16] * SCL_P
        r += 1
    for i in range(32):                         # squares
        Brows[r] = -0.5 * A[:, i, i] * SCL_D
        r += 1
    for i in range(32):                         # linear
        Brows[r] = m[:, i] * SCL_L
        r += 1
    f8 = ml_dtypes.float8_e4m3
    b0 = bias.astype(f8).astype(np.float64)
    b1 = (bias - b0).astype(f8).astype(np.float64)
    Brows[r] = b0
    Brows[r + 1] = b1
    Brows[r + 2] = bias - b0 - b1
    # row r+3 stays zero (pad)
    return Brows.astype(np.float32), np.float32(-C)


def _host_lift(pts):
    """fp8 lift planes [NROWS, npts] for one core's points [npts, 32]."""
    X = np.ascontiguousarray(pts.T)             # [32, npts]
    npts = X.shape[1]
    L = np.empty((NROWS, npts), np.float32)
    r = 0
    inv_p = np.float32(1.0 / SCL_P)
    for o in range(1, 16):
        L[r:r + 32] = X * np.roll(X, -o, axis=0) * inv_p
        r += 32
    L[r:r + 16] = X[:16] * X[16:] * inv_p
    r += 16
    L[r:r + 32] = X * X * np.float32(1.0 / SCL_D)
    r += 32
    L[r:r + 32] = X * np.float32(1.0 / SCL_L)
    r += 32
    L[r:r + 2] = 1.0
    L[r + 2] = 1.0
    L[r + 3] = 0.0
    return L


def kernel(points, centers, covs_inv_sqrt, weights, threshold):
    points = np.asarray(points, dtype=np.float32)
    Brows, negC = _host_params(np.asarray(centers),
                               np.asarray(covs_inv_sqrt),
                               np.asarray(weights), np.asarray(threshold))
    f8 = ml_dtypes.float8_e4m3
    B8 = Brows.astype(f8)
    bab = np.ascontiguousarray(
        B8[:512].reshape(4, 128, K).transpose(1, 0, 2))
    bc = np.ascontiguousarray(
        B8[512:].reshape(2, CROWS, K).transpose(1, 0, 2))
    consts = np.full((128, 1), negC, dtype=np.float32)

    in_maps = []
    for r in range(NCORES):
        L8 = _host_lift(points[r * NC_PTS:(r + 1) * NC_PTS]).astype(f8)
        lab = np.ascontiguousarray(
            L8[:512].reshape(4, 128, NC_PTS).transpose(1, 0, 2))
        lc = np.ascontiguousarray(
            L8[512:].reshape(2, CROWS, NC_PTS).transpose(1, 0, 2))
        in_maps.append({"lab": lab, "lc": lc, "bab": bab, "bc": bc,
                        "consts": consts})

    nc = _get_module()
    res = bass_utils.run_bass_kernel_spmd(nc, in_maps,
                                          core_ids=list(range(NCORES)))
    ll = np.concatenate([res.results[r]["out"].T.reshape(-1)
                         for r in range(NCORES)])
    return ll.reshape(N, 1).astype(np.float32)
